# revision 62
# baseline (speedup 1.0000x reference)
"""Trainium2 Bass kernel for nn_BlockWiseDistanceComputation (hyperbolic /
Poincare block-sparse attention), 8-core head-parallel.

Math per head h (B=1, N=2048, D=512, H=8, Dh=64, BM=BN=128, causal):
  q = x@Wq_h, k = x@Wk_h, v = x@Wv_h        (column-parallel slices)
  diff = qn[m] + kn[n] - 2 q.k
  arg  = 1 + 2c*diff/((1-c qn)(1-c kn)),  clipped to >= 1+EPS
  s    = -arccosh(arg)/sqrt(c)
  block softmax with per-block max (own max, no cross-block rescale)
  out_h = (e @ v)/sum(e);  partial = out_h @ Wo_h   (row-parallel)
Host sums the 8 partials and adds bo.

Key device trick (c == 1, verified at call time): with g = arg+sqrt(arg^2-1)
= exp(arccosh(arg)), exp(s - bmax) = gmin/g = h/hmax where h = 1/g =
arg - sqrt(arg^2-1) in closed form -- no exp/ln needed; the whole elementwise
chain fits one ACT table set (relu/square/sqrt/copy).

Device works with hn = -(1+t) + u  (t = psum*f scale, u = sqrt((1+t)^2-1));
hn = -h is strictly in (-1, 0); per-block min of hn is -hmax and
e = hn * (1/hnmin) >= 0. Masked elements of diagonal blocks are zeroed in hn
(excluded from the min since hn < 0 strictly), giving e = 0 exactly.

The matmul-heavy paths run in f16 (inputs rounded to f16, f32 PSUM
accumulate): S scores, eT transpose-with-diag, PV, projections, out-proj.
"""

import numpy as np
from contextlib import ExitStack

import concourse.bass as bass
import concourse.bacc as bacc
import concourse.tile as tile
from concourse import mybir
from concourse.bass_utils import run_bass_kernel_spmd

AF = mybir.ActivationFunctionType
ALU = mybir.AluOpType
F32 = mybir.dt.float32
F16 = mybir.dt.float16
BF16 = mybir.dt.bfloat16

H, N, D, DH, P = 8, 2048, 512, 64, 128
NT = N // P          # 16 row/col tiles
KC = D // P          # 4 contraction chunks
KV = DH + 65         # merged k|v_aug rhs width (64 + 65)
EPS = 1e-6
CLIP2 = (1.0 + EPS) ** 2 - 1.0   # clip floor for (1+t)^2 - 1

DEFAULT_CFG = dict(
    s_dt="f16",        # score matmul dtype
    proj_dt="f16",     # q/k/v projection matmul dtype
    et_dt="f16",       # e-transpose (diag matmul) dtype
    pv_dt="f16",       # PV matmul dtype
    op_dt="f16",       # out-proj matmul dtype
    clip_path="relu_row",  # 'relu_row' | 'relu' (ACT) | 'dve' | 'gpsimd'
    etcopy_engine="vector",
    xtcopy_engine="vector",
    hp_bufs=3, cw_bufs=3, et_bufs=3, row_bufs=3,
    x16=True,          # pre-cast x to f16 on gpsimd; f16 x-transposes
    hmin16=True,       # f16 hmin (2x DVE reduce)
    has_bq=False, has_bkv=False,
)


def _sbdt(dt_str):
    return {"f16": F16, "bf16": BF16}.get(dt_str, F32)


def _npdt(dt_str):
    import ml_dtypes
    return {"f16": np.float16, "bf16": ml_dtypes.bfloat16}.get(
        dt_str, np.float32)


def build_program(cfg=None):
    """Build the single-core SPMD bass program. Returns (nc, input_names)."""
    cfg = {**DEFAULT_CFG, **(cfg or {})}
    nc = bacc.Bacc("TRN2", debug=False, num_devices=8)
    s_dt = _sbdt(cfg["s_dt"])
    pj_dt = _sbdt(cfg["proj_dt"])
    hp_dt = _sbdt(cfg["et_dt"])
    pv_dt = _sbdt(cfg["pv_dt"])
    op_dt = _sbdt(cfg["op_dt"])

    xt_d = nc.dram_tensor("xt", [D, N], pj_dt, kind="ExternalInput").ap()
    wq_d = nc.dram_tensor("wq", [D, DH], pj_dt, kind="ExternalInput").ap()
    wkv_d = nc.dram_tensor("wkv", [D, DH + 65], pj_dt,
                           kind="ExternalInput").ap()
    wo_d = nc.dram_tensor("wo", [DH, D], op_dt, kind="ExternalInput").ap()
    id_d = nc.dram_tensor("ident", [P, P], F32, kind="ExternalInput").ap()
    tm_d = nc.dram_tensor("trimask", [P, P], hp_dt,
                          kind="ExternalInput").ap()
    names = ["xt", "wq", "wkv", "wo", "ident", "trimask"]
    if cfg["has_bkv"]:
        bkv_d = nc.dram_tensor("bkvrow", [1, KV], pj_dt,
                               kind="ExternalInput").ap()
        names.append("bkvrow")
    if cfg["has_bq"]:
        bq_d = nc.dram_tensor("bqcol", [DH, 1], F32, kind="ExternalInput").ap()
        names.append("bqcol")
    out_d = nc.dram_tensor("out", [N, D], F32, kind="ExternalOutput").ap()

    with tile.TileContext(nc) as tc, ExitStack() as ctx:
        # ---- persistent SBUF ----
        per = ctx.enter_context(tc.tile_pool(name="per", bufs=1))
        ident = per.tile([P, P], F32, tag="ident")
        nc.sync.dma_start(ident[:], id_d)
        idents = {F32: ident}
        for want in {s_dt, op_dt, pj_dt} - {F32}:
            idw = per.tile([P, P], want, tag=f"id{want}", name=f"id{want}")
            nc.vector.tensor_copy(idw[:], ident[:])
            idents[want] = idw
        trimask = per.tile([P, P], hp_dt, tag="trimask")
        nc.sync.dma_start(trimask[:], tm_d)
        xT = [per.tile([P, N], pj_dt, tag=f"xT{j}", name=f"xT{j}")
              for j in range(KC)]
        laug = per.tile([DH + 2, N], s_dt, tag="laug")  # [q^T; qn; 1]
        raug = per.tile([DH + 2, N], s_dt, tag="raug")  # [-2rb k^T; rb; rb kn]
        vaug = per.tile([P, NT * (DH + 1)], pv_dt, tag="vaug")
        wq_sb = per.tile([P, KC * DH], pj_dt, tag="wq")
        wkv_sb = per.tile([P, KC * KV], pj_dt, tag="wkv")
        wo_sb = per.tile([DH, D], op_dt, tag="wo")
        if cfg["has_bkv"]:
            bkv_sb = per.tile([1, KV], pj_dt, tag="bkv")
            ones1 = per.tile([1, P], pj_dt, tag="ones1")
            nc.vector.memset(ones1[:], 1.0)
            nc.sync.dma_start(bkv_sb[:], bkv_d)
        ones64 = per.tile([DH, 1], pj_dt, tag="ones64")
        negone = per.tile([P, 1], F32, tag="negone")
        meps = per.tile([P, 1], F32, tag="meps")
        peps = per.tile([P, 1], F32, tag="peps")
        qsq = per.tile([DH, N], pj_dt, tag="qsq")
        kn_col = per.tile([P, NT], F32, tag="kn_col")
        sc_pack = per.tile([P, 2 * NT], F32, tag="sc_pack")  # s1 | s2
        rb2_col = per.tile([P, NT], F32, tag="rb2")
        zb_col = per.tile([P, NT], F32, tag="zb")
        fneg_col = per.tile([P, NT], F32, tag="fneg")
        fpos_col = per.tile([P, NT], F32, tag="fpos")
        zq_col = per.tile([P, NT], F32, tag="zq")
        qn16 = per.tile([NT, P], s_dt, tag="qn16")

        nc.vector.memset(negone[:], -1.0)
        nc.vector.memset(meps[:], -EPS)
        nc.vector.memset(peps[:], 1.0 + EPS)
        nc.vector.memset(ones64[:], 1.0)
        for c in range(KC):
            nc.sync.dma_start(wq_sb[:, c * DH:(c + 1) * DH],
                              wq_d[c * P:(c + 1) * P, :])
            nc.sync.dma_start(wkv_sb[:, c * KV:(c + 1) * KV],
                              wkv_d[c * P:(c + 1) * P, :])
        nc.sync.dma_start(wo_sb[:], wo_d)
        if cfg["has_bq"]:
            bq_sb = per.tile([DH, 1], F32, tag="bq")
            nc.sync.dma_start(bq_sb[:], bq_d)

        def psum_copy(engine, dst, src):
            if engine == "vector":
                nc.vector.tensor_copy(dst, src)
            else:
                nc.scalar.activation(dst, src, AF.Copy)

        # ---- phase A: x^T arrives pre-transposed from the host ----
        for j in range(KC):
            nc.sync.dma_start(xT[j][:], xt_d[j * P:(j + 1) * P, :])

        # ---- phase B1: merged K|V projections, K-side stats ----
        has_bkv = cfg["has_bkv"]
        with tc.tile_pool(name="kb", bufs=3) as kb, \
             tc.tile_pool(name="psk", bufs=3, space="PSUM") as psk, \
             tc.tile_pool(name="pskt", bufs=2, space="PSUM") as pskt:
            if not has_bkv:  # ones column of v_aug written directly
                nc.vector.memset(
                    vaug[:].rearrange("p (t c) -> p t c", c=DH + 1)[:, :, DH:DH + 1],
                    1.0)
            for g in range(NT // 4):
                ps_t = pskt.tile([DH, 4 * P], s_dt, tag="pskt")
                for q4 in range(4):
                    i = g * 4 + q4
                    ps_kv = psk.tile([P, KV], F32, tag="psk")
                    for c in range(KC):
                        nc.tensor.matmul(
                            ps_kv[:], xT[c][:, i * P:(i + 1) * P],
                            wkv_sb[:, c * KV:(c + 1) * KV],
                            start=(c == 0),
                            stop=(c == KC - 1 and not has_bkv))
                    if has_bkv:
                        nc.tensor.matmul(ps_kv[:], ones1[:], bkv_sb[:],
                                         start=False, stop=True)
                        nc.scalar.activation(
                            vaug[:, i * (DH + 1):(i + 1) * (DH + 1)],
                            ps_kv[:, DH:KV], AF.Copy)
                    else:
                        nc.scalar.activation(
                            vaug[:, i * (DH + 1):i * (DH + 1) + DH],
                            ps_kv[:, DH:DH + DH], AF.Copy)
                    ksq = kb.tile([P, DH], F32, tag="ksq")
                    nc.scalar.activation(ksq[:], ps_kv[:, 0:DH], AF.Square)
                    nc.vector.tensor_reduce(
                        kn_col[:, i:i + 1], ksq[:],
                        axis=mybir.AxisListType.X, op=ALU.add)
                    nc.vector.tensor_scalar(
                        zb_col[:, i:i + 1], kn_col[:, i:i + 1], -1.0, 1.0,
                        ALU.mult, ALU.add)
                    nc.vector.reciprocal(sc_pack[:, i:i + 1],
                                         zb_col[:, i:i + 1])
                    nc.vector.tensor_tensor(
                        sc_pack[:, NT + i:NT + i + 1], sc_pack[:, i:i + 1],
                        kn_col[:, i:i + 1], op=ALU.mult)
                    nc.vector.tensor_scalar_mul(
                        rb2_col[:, i:i + 1], sc_pack[:, i:i + 1], -2.0)
                    kp = kb.tile([P, DH], s_dt, tag="kp")
                    nc.scalar.activation(kp[:], ps_kv[:, 0:DH], AF.Copy,
                                         scale=rb2_col[:, i:i + 1])
                    nc.tensor.transpose(ps_t[:, q4 * P:(q4 + 1) * P],
                                        kp[:], idents[s_dt][:])
                nc.vector.tensor_copy(raug[0:DH, g * 4 * P:(g + 1) * 4 * P],
                                      ps_t[:])
            # stat rows: transpose sc_pack -> [32,128] -> SBUF -> DMA to raug
            ps_sc = pskt.tile([2 * NT, P], F32, tag="pssc")
            nc.tensor.transpose(ps_sc[:], sc_pack[:], ident[:])
            sc_t = kb.tile([2 * NT, P], s_dt, tag="sc_t")
            nc.vector.tensor_copy(sc_t[:], ps_sc[:])
            nc.sync.dma_start(
                raug[DH:DH + 2, :].rearrange("p (t n) -> p t n", n=P),
                sc_t[:])

        # ---- phase B2: Q side (wide) ----
        with tc.tile_pool(name="psq", bufs=2, space="PSUM") as psq, \
             tc.tile_pool(name="psqn", bufs=2, space="PSUM") as psqn, \
             tc.tile_pool(name="qtmp", bufs=1) as qtmp:
            CH = 512
            qnrow = qtmp.tile([2, N], s_dt, tag="qnrow")
            nc.vector.memset(qnrow[:], 1.0)
            for ch in range(N // CH):
                ps_q = psq.tile([DH, CH], F32, tag="psq")
                for c in range(KC):
                    nc.tensor.matmul(
                        ps_q[:], wq_sb[:, c * DH:(c + 1) * DH],
                        xT[c][:, ch * CH:(ch + 1) * CH],
                        start=(c == 0), stop=(c == KC - 1))
                if cfg["has_bq"]:
                    nc.scalar.activation(laug[0:DH, ch * CH:(ch + 1) * CH],
                                         ps_q[:], AF.Identity,
                                         bias=bq_sb[:, 0:1])
                    nc.scalar.activation(qsq[:, ch * CH:(ch + 1) * CH],
                                         ps_q[:], AF.Square,
                                         bias=bq_sb[:, 0:1])
                else:
                    nc.scalar.activation(laug[0:DH, ch * CH:(ch + 1) * CH],
                                         ps_q[:], AF.Copy)
                    nc.scalar.activation(qsq[:, ch * CH:(ch + 1) * CH],
                                         ps_q[:], AF.Square)
                ps_n = psqn.tile([1, CH], F32, tag="psqn")
                nc.tensor.matmul(ps_n[:], ones64[:],
                                 qsq[:, ch * CH:(ch + 1) * CH],
                                 start=True, stop=True)
                nc.vector.tensor_copy(qnrow[0:1, ch * CH:(ch + 1) * CH],
                                      ps_n[:])
            nc.sync.dma_start(laug[DH:DH + 2, :], qnrow[:])
            # qn col-form -> fneg scales
            nc.sync.dma_start(qn16[:], qnrow[0:1, :])
            ps_qc = psqn.tile([P, NT], s_dt, tag="psqc")
            nc.tensor.transpose(ps_qc[:], qn16[:], idents[s_dt][0:NT, 0:NT])
            nc.vector.tensor_scalar(zq_col[:], ps_qc[:], -1.0, 1.0,
                                    ALU.mult, ALU.add)
            nc.vector.reciprocal(fpos_col[:], zq_col[:])
            nc.vector.tensor_scalar_mul(fneg_col[:], fpos_col[:], -2.0)
            nc.vector.tensor_scalar_mul(fpos_col[:], fpos_col[:], 2.0)

        # ---- phase C: rows ----
        clip = cfg["clip_path"]
        hm_dt = F16 if cfg["hmin16"] else F32
        with tc.tile_pool(name="hp", bufs=cfg["hp_bufs"]) as hpp, \
             tc.tile_pool(name="cwrow", bufs=cfg["row_bufs"]) as cwrow, \
             tc.tile_pool(name="cw", bufs=cfg["cw_bufs"]) as cw, \
             tc.tile_pool(name="etp", bufs=cfg["et_bufs"]) as etp, \
             tc.tile_pool(name="stat", bufs=2) as statp, \
             tc.tile_pool(name="pss", bufs=3, space="PSUM") as pss, \
             tc.tile_pool(name="pset", bufs=2, space="PSUM") as pset, \
             tc.tile_pool(name="pso", bufs=2, space="PSUM") as pso, \
             tc.tile_pool(name="psop", bufs=1, space="PSUM") as psop:
            for r in range(NT):
                W = (r + 1) * P
                hp = hpp.tile([P, N], hp_dt, tag="hp")
                hmin = statp.tile([P, NT], hm_dt, tag="hmin")
                f_ap = fneg_col[:, r:r + 1]
                fp_ap = fpos_col[:, r:r + 1]
                if clip == "relu_row":
                    # Relu per chunk (frees PSUM), then row-wide passes.
                    tr = cwrow.tile([P, N], F32, tag="tr")
                    o = 0
                    while o < W:
                        w = min(512, W - o)
                        ps_s = pss.tile([P, 512], F32, tag="pss")
                        nc.tensor.matmul(
                            ps_s[:, 0:w], laug[:, r * P:(r + 1) * P],
                            raug[:, o:o + w], start=True, stop=True)
                        nc.scalar.activation(tr[:, o:o + w], ps_s[:, 0:w],
                                             AF.Relu, bias=meps[:, 0:1],
                                             scale=fp_ap)
                        o += w
                    sq = cwrow.tile([P, N], F32, tag="sqr")
                    nc.scalar.activation(sq[:, 0:W], tr[:, 0:W], AF.Square,
                                         bias=peps[:, 0:1])
                    u_t = cwrow.tile([P, N], F32, tag="ur")
                    nc.scalar.activation(u_t[:, 0:W], sq[:, 0:W], AF.Sqrt,
                                         bias=negone[:, 0:1])
                    # hn = (tr*-1 - (1+eps)) + u  (uses clipped t; diff ~1e-6)
                    nc.vector.affine_then_add(hp[:, 0:W], tr[:, 0:W],
                                              u_t[:, 0:W], -1.0,
                                              -(1.0 + EPS))
                    nc.vector.tensor_tensor(hp[:, W - P:W], hp[:, W - P:W],
                                            trimask[:], op=ALU.mult)
                    nc.vector.tensor_reduce(
                        hmin[:, 0:r + 1],
                        hp[:, 0:W].rearrange("p (b n) -> p b n", n=P),
                        axis=mybir.AxisListType.X, op=ALU.min)
                else:
                  o = 0
                  while o < W:
                    w = min(512, W - o)
                    ps_s = pss.tile([P, 512], F32, tag="pss")
                    nc.tensor.matmul(
                        ps_s[:, 0:w], laug[:, r * P:(r + 1) * P],
                        raug[:, o:o + w], start=True, stop=True)
                    if clip == "relu":
                        tr = cw.tile([P, 512], F32, tag="tr")
                        nc.scalar.activation(tr[:, 0:w], ps_s[:, 0:w],
                                             AF.Relu, bias=meps[:, 0:1],
                                             scale=fp_ap)
                        sq = cw.tile([P, 512], F32, tag="sq")
                        nc.scalar.activation(sq[:, 0:w], tr[:, 0:w],
                                             AF.Square, bias=peps[:, 0:1])
                        u_in = sq
                    else:
                        sq = cw.tile([P, 512], F32, tag="sq")
                        nc.scalar.activation(sq[:, 0:w], ps_s[:, 0:w],
                                             AF.Square, bias=negone[:, 0:1],
                                             scale=f_ap)
                        d_t = cw.tile([P, 512], F32, tag="d")
                        eng = (nc.gpsimd if clip == "gpsimd" else nc.vector)
                        eng.tensor_scalar(d_t[:, 0:w], sq[:, 0:w], -1.0,
                                          CLIP2, ALU.add, ALU.max)
                        u_in = d_t
                    u_t = cw.tile([P, 512], F32, tag="u")
                    nc.scalar.activation(u_t[:, 0:w], u_in[:, 0:w], AF.Sqrt,
                                         bias=(negone[:, 0:1]
                                               if clip == "relu" else 0.0))
                    nc.vector.affine_then_add(hp[:, o:o + w], ps_s[:, 0:w],
                                              u_t[:, 0:w], f_ap, -1.0)
                    if o + w == W:  # diagonal block: zero strict upper tri
                        nc.vector.tensor_tensor(hp[:, W - P:W], hp[:, W - P:W],
                                                trimask[:], op=ALU.mult)
                    nb = w // P
                    nc.vector.tensor_reduce(
                        hmin[:, o // P:o // P + nb],
                        hp[:, o:o + w].rearrange("p (b n) -> p b n", n=P),
                        axis=mybir.AxisListType.X, op=ALU.min)
                    o += w
                rh = statp.tile([P, NT], F32, tag="rh")
                nc.vector.reciprocal(rh[:, 0:r + 1], hmin[:, 0:r + 1])
                ps_o = pso.tile([P, DH + 1], F32, tag="pso")
                nblk = r + 1
                for g in range((nblk + 3) // 4):
                    c0, c1 = g * 4, min(g * 4 + 4, nblk)
                    ps_et = pset.tile([P, 512], F32, tag="pset")
                    et_sb = etp.tile([P, 512], pv_dt, tag="et")
                    for c in range(c0, c1):
                        q = c - c0
                        diag = cw.tile([P, P], hp_dt, tag="diag")
                        nc.gpsimd.affine_select(
                            diag[:], rh[:, c:c + 1].broadcast_to([P, P]),
                            pattern=[[1, P]], compare_op=ALU.is_equal,
                            fill=0.0, base=0, channel_multiplier=-1)
                        nc.tensor.matmul(
                            ps_et[:, q * P:(q + 1) * P],
                            hp[:, c * P:(c + 1) * P], diag[:],
                            start=True, stop=True)
                    wg = (c1 - c0) * P
                    psum_copy(cfg["etcopy_engine"], et_sb[:, 0:wg],
                              ps_et[:, 0:wg])
                    for c in range(c0, c1):
                        q = c - c0
                        nc.tensor.matmul(
                            ps_o[:], et_sb[:, q * P:(q + 1) * P],
                            vaug[:, c * (DH + 1):(c + 1) * (DH + 1)],
                            start=(c == 0), stop=(c == nblk - 1),
                            skip_group_check=True)
                # out-proj on UNNORMALIZED o; 1/norm folded into final copy
                # (per-row scale commutes out of the Dh contraction)
                ns = statp.tile([P, 1], F32, tag="ns")
                nc.vector.tensor_scalar_max(ns[:], ps_o[:, DH:DH + 1], EPS)
                rn = statp.tile([P, 1], F32, tag="rn")
                nc.vector.reciprocal(rn[:], ns[:])
                o_sb = statp.tile([P, DH], op_dt, tag="o_sb")
                nc.scalar.activation(o_sb[:], ps_o[:, 0:DH], AF.Copy)
                ps_ot = pset.tile([DH, P], op_dt, tag="pset")
                nc.tensor.transpose(ps_ot[:], o_sb[:], idents[op_dt][:])
                ot_sb = statp.tile([DH, P], op_dt, tag="ot")
                nc.vector.tensor_copy(ot_sb[:], ps_ot[:])
                ps_op = psop.tile([P, D], F32, tag="psop")
                nc.tensor.matmul(ps_op[:], ot_sb[:], wo_sb[:],
                                 start=True, stop=True)
                op_sb = etp.tile([P, D], F32, tag="op_sb")
                nc.scalar.activation(op_sb[:], ps_op[:], AF.Copy, scale=rn[:])
                nc.sync.dma_start(out_d[r * P:(r + 1) * P, :], op_sb[:])

    nc.compile()
    return nc, names


def _host_fallback(x, c, Wq, bq, Wk, bk, Wv, bv, Wo, bo):
    """Numpy replica of the reference for inputs outside the specialized
    regime (never hit for the shipped setup_inputs). Kept for safety."""
    B, N_, D_ = x.shape
    Dh = D_ // H
    cc = np.maximum(np.abs(c), 1e-6)[0]
    sqrt_c = np.sqrt(max(cc, EPS))
    x2 = x.reshape(N_, D_)

    def proj(W, b):
        return (x2 @ W + b).reshape(N_, H, Dh).transpose(1, 0, 2)

    q, k, v = proj(Wq, bq), proj(Wk, bk), proj(Wv, bv)
    qn = (q ** 2).sum(-1)
    kn = (k ** 2).sum(-1)
    out = np.zeros((H, N_, Dh), np.float32)
    BM = P
    for h in range(H):
        qk = q[h] @ k[h].T
        diff = np.clip(qn[h][:, None] + kn[h][None, :] - 2 * qk, 0, None)
        den = np.clip((1 - cc * qn[h])[:, None] * (1 - cc * kn[h])[None, :],
                      EPS, None)
        arg = np.clip(1 + 2 * cc * diff / den, 1 + EPS, None)
        s = -np.arccosh(arg) / sqrt_c
        nbm = N_ // BM
        tri = np.triu(np.ones((BM, BM), bool), 1)
        e = np.zeros_like(s)
        for r in range(nbm):
            for cb in range(r + 1):
                blk = s[r * BM:(r + 1) * BM, cb * BM:(cb + 1) * BM].copy()
                m = tri if cb == r else np.zeros((BM, BM), bool)
                bm = np.where(m, -np.inf, blk).max(axis=1, keepdims=True)
                bm = np.where(np.isfinite(bm), bm, 0.0)
                eb = np.where(m, 0.0, np.exp(blk - bm))
                e[r * BM:(r + 1) * BM, cb * BM:(cb + 1) * BM] = eb
        norm = np.clip(e.sum(axis=1), EPS, None)
        out[h] = (e @ v[h]) / norm[:, None]
    full = out.transpose(1, 0, 2).reshape(N_, D_)
    return (full @ Wo + bo).reshape(B, N_, D_).astype(np.float32)


_PROG_CACHE = {}


def _get_program(cfg_key, cfg):
    if cfg_key not in _PROG_CACHE:
        _PROG_CACHE[cfg_key] = build_program(cfg)
    return _PROG_CACHE[cfg_key]


def make_in_maps(x, Wq, bq, Wk, bk, Wv, bv, Wo, cfg=None):
    cfg = {**DEFAULT_CFG, **(cfg or {})}
    np_pj = _npdt(cfg["proj_dt"])
    np_op = _npdt(cfg["op_dt"])
    np_tm = _npdt(cfg["et_dt"])
    xt = np.ascontiguousarray(
        x.reshape(N, D).astype(np.float32).T).astype(np_pj)
    ident = np.eye(P, dtype=np.float32)
    trimask = np.tril(np.ones((P, P), np.float32)).astype(np_tm)
    in_maps = []
    for h in range(H):
        sl = slice(h * DH, (h + 1) * DH)
        wkv = np.zeros((D, KV), np.float32)
        wkv[:, :DH] = Wk[:, sl]
        wkv[:, DH:DH + DH] = Wv[:, sl]
        m = {
            "xt": xt,
            "wq": np.ascontiguousarray(Wq[:, sl]).astype(np_pj),
            "wkv": wkv.astype(np_pj),
            "wo": np.ascontiguousarray(Wo[sl, :]).astype(np_op),
            "ident": ident,
            "trimask": trimask,
        }
        if cfg["has_bkv"]:
            bkv = np.zeros((1, KV), np.float32)
            bkv[0, :DH] = bk[sl]
            bkv[0, DH:DH + DH] = bv[sl]
            bkv[0, DH + DH] = 1.0
            m["bkvrow"] = bkv.astype(np_pj)
        if cfg["has_bq"]:
            m["bqcol"] = np.ascontiguousarray(
                bq[sl].reshape(DH, 1)).astype(np.float32)
        in_maps.append(m)
    return in_maps


def run_device(x, Wq, bq, Wk, bk, Wv, bv, Wo, cfg=None, trace=False,
               tmpdir=None):
    cfg_full = {**DEFAULT_CFG, **(cfg or {})}
    cfg_full["has_bq"] = bool(np.any(bq))
    cfg_full["has_bkv"] = bool(np.any(bk)) or bool(np.any(bv))
    cfg_key = tuple(sorted(cfg_full.items()))
    nc, _ = _get_program(cfg_key, cfg_full)
    in_maps = make_in_maps(x, Wq, bq, Wk, bk, Wv, bv, Wo, cfg_full)
    res = run_bass_kernel_spmd(nc, in_maps, core_ids=list(range(H)),
                               trace=trace, tmpdir=tmpdir)
    partial = np.zeros((N, D), np.float64)
    for rm in res.results:
        partial += rm["out"].astype(np.float64)
    return partial, res


def kernel(x, c, Wq, bq, Wk, bk, Wv, bv, Wo, bo):
    x = np.asarray(x); c = np.asarray(c)
    Wq = np.asarray(Wq, np.float32); bq = np.asarray(bq, np.float32)
    Wk = np.asarray(Wk, np.float32); bk = np.asarray(bk, np.float32)
    Wv = np.asarray(Wv, np.float32); bv = np.asarray(bv, np.float32)
    Wo = np.asarray(Wo, np.float32); bo = np.asarray(bo, np.float32)

    cc = max(abs(float(c.reshape(-1)[0])), 1e-6)
    if abs(np.sqrt(max(cc, EPS)) - 1.0) > 1e-9:
        return _host_fallback(x, c, Wq, bq, Wk, bk, Wv, bv, Wo, bo)

    partial, _ = run_device(x, Wq, bq, Wk, bk, Wv, bv, Wo)
    out = (partial + bo.astype(np.float64)).astype(np.float32)
    return out.reshape(1, N, D)


# revision 66
# speedup vs baseline: 1.1813x; 1.1813x over previous
"""Trainium2 Bass kernel for nn_BlockWiseDistanceComputation (hyperbolic /
Poincare block-sparse attention), 8-core head-parallel.

Math per head h (B=1, N=2048, D=512, H=8, Dh=64, BM=BN=128, causal):
  q = x@Wq_h, k = x@Wk_h, v = x@Wv_h        (column-parallel slices)
  diff = qn[m] + kn[n] - 2 q.k
  arg  = 1 + 2c*diff/((1-c qn)(1-c kn)),  clipped to >= 1+EPS
  s    = -arccosh(arg)/sqrt(c)
  block softmax with per-block max (own max, no cross-block rescale)
  out_h = (e @ v)/sum(e);  partial = out_h @ Wo_h   (row-parallel)
Host sums the 8 partials and adds bo.

Key device trick (c == 1, verified at call time): with g = arg+sqrt(arg^2-1)
= exp(arccosh(arg)), exp(s - bmax) = gmin/g = h/hmax where h = 1/g =
arg - sqrt(arg^2-1) in closed form -- no exp/ln needed; the whole elementwise
chain fits one ACT table set (relu/square/sqrt/copy).

Device works with hn = -(1+t) + u  (t = psum*f scale, u = sqrt((1+t)^2-1));
hn = -h is strictly in (-1, 0); per-block min of hn is -hmax and
e = hn * (1/hnmin) >= 0. Masked elements of diagonal blocks are zeroed in hn
(excluded from the min since hn < 0 strictly), giving e = 0 exactly.

The matmul-heavy paths run in f16 (inputs rounded to f16, f32 PSUM
accumulate): S scores, eT transpose-with-diag, PV, projections, out-proj.
"""

import numpy as np
from contextlib import ExitStack

import concourse.bass as bass
import concourse.bacc as bacc
import concourse.tile as tile
from concourse import mybir
from concourse.bass_utils import run_bass_kernel_spmd

AF = mybir.ActivationFunctionType
ALU = mybir.AluOpType
F32 = mybir.dt.float32
F16 = mybir.dt.float16
BF16 = mybir.dt.bfloat16

H, N, D, DH, P = 8, 2048, 512, 64, 128
NT = N // P          # 16 row/col tiles
KC = D // P          # 4 contraction chunks
KV = DH + 65         # merged k|v_aug rhs width (64 + 65)
EPS = 1e-6
CLIP2 = (1.0 + EPS) ** 2 - 1.0   # clip floor for (1+t)^2 - 1

DEFAULT_CFG = dict(
    s_dt="f16",        # score matmul dtype
    proj_dt="f16",     # q/k/v projection matmul dtype
    et_dt="f16",       # e-transpose (diag matmul) dtype
    pv_dt="f16",       # PV matmul dtype
    op_dt="f16",       # out-proj matmul dtype
    clip_path="relu_row",  # 'relu_row' | 'relu' (ACT) | 'dve' | 'gpsimd'
    etcopy_engine="vector",
    xtcopy_engine="vector",
    hp_bufs=3, cw_bufs=3, et_bufs=3, row_bufs=3,
    pipeline=True,     # software-pipeline stage2(r-1) after stage1(r)
    hmin16=True,       # f16 hmin (2x DVE reduce)
    has_bq=False, has_bkv=False,
)


def _sbdt(dt_str):
    return {"f16": F16, "bf16": BF16}.get(dt_str, F32)


def _npdt(dt_str):
    import ml_dtypes
    return {"f16": np.float16, "bf16": ml_dtypes.bfloat16}.get(
        dt_str, np.float32)


def build_program(cfg=None):
    """Build the single-core SPMD bass program. Returns (nc, input_names)."""
    cfg = {**DEFAULT_CFG, **(cfg or {})}
    nc = bacc.Bacc("TRN2", debug=False, num_devices=8)
    s_dt = _sbdt(cfg["s_dt"])
    pj_dt = _sbdt(cfg["proj_dt"])
    hp_dt = _sbdt(cfg["et_dt"])
    pv_dt = _sbdt(cfg["pv_dt"])
    op_dt = _sbdt(cfg["op_dt"])

    xt_d = nc.dram_tensor("xt", [D, N], pj_dt, kind="ExternalInput").ap()
    wq_d = nc.dram_tensor("wq", [D, DH], pj_dt, kind="ExternalInput").ap()
    wkv_d = nc.dram_tensor("wkv", [D, DH + 65], pj_dt,
                           kind="ExternalInput").ap()
    wo_d = nc.dram_tensor("wo", [DH, D], op_dt, kind="ExternalInput").ap()
    id_d = nc.dram_tensor("ident", [P, P], F32, kind="ExternalInput").ap()
    tm_d = nc.dram_tensor("trimask", [P, P], hp_dt,
                          kind="ExternalInput").ap()
    names = ["xt", "wq", "wkv", "wo", "ident", "trimask"]
    if cfg["has_bkv"]:
        bkv_d = nc.dram_tensor("bkvrow", [1, KV], pj_dt,
                               kind="ExternalInput").ap()
        names.append("bkvrow")
    if cfg["has_bq"]:
        bq_d = nc.dram_tensor("bqcol", [DH, 1], F32, kind="ExternalInput").ap()
        names.append("bqcol")
    out_d = nc.dram_tensor("out", [N, D], F32, kind="ExternalOutput").ap()

    with tile.TileContext(nc) as tc, ExitStack() as ctx:
        # ---- persistent SBUF ----
        per = ctx.enter_context(tc.tile_pool(name="per", bufs=1))
        ident = per.tile([P, P], F32, tag="ident")
        nc.sync.dma_start(ident[:], id_d)
        idents = {F32: ident}
        for want in {s_dt, op_dt, pj_dt} - {F32}:
            idw = per.tile([P, P], want, tag=f"id{want}", name=f"id{want}")
            nc.vector.tensor_copy(idw[:], ident[:])
            idents[want] = idw
        trimask = per.tile([P, P], hp_dt, tag="trimask")
        nc.sync.dma_start(trimask[:], tm_d)
        xT = [per.tile([P, N], pj_dt, tag=f"xT{j}", name=f"xT{j}")
              for j in range(KC)]
        laug = per.tile([DH + 2, N], s_dt, tag="laug")  # [q^T; qn; 1]
        raug = per.tile([DH + 2, N], s_dt, tag="raug")  # [-2rb k^T; rb; rb kn]
        vaug = per.tile([P, NT * (DH + 1)], pv_dt, tag="vaug")
        wq_sb = per.tile([P, KC * DH], pj_dt, tag="wq")
        wkv_sb = per.tile([P, KC * KV], pj_dt, tag="wkv")
        wo_sb = per.tile([DH, D], op_dt, tag="wo")
        if cfg["has_bkv"]:
            bkv_sb = per.tile([1, KV], pj_dt, tag="bkv")
            ones1 = per.tile([1, P], pj_dt, tag="ones1")
            nc.vector.memset(ones1[:], 1.0)
            nc.sync.dma_start(bkv_sb[:], bkv_d)
        ones64 = per.tile([DH, 1], pj_dt, tag="ones64")
        negone = per.tile([P, 1], F32, tag="negone")
        meps = per.tile([P, 1], F32, tag="meps")
        peps = per.tile([P, 1], F32, tag="peps")
        qsq = per.tile([DH, N], pj_dt, tag="qsq")
        kn_col = per.tile([P, NT], F32, tag="kn_col")
        sc_pack = per.tile([P, 2 * NT], F32, tag="sc_pack")  # s1 | s2
        rb2_col = per.tile([P, NT], F32, tag="rb2")
        zb_col = per.tile([P, NT], F32, tag="zb")
        fneg_col = per.tile([P, NT], F32, tag="fneg")
        fpos_col = per.tile([P, NT], F32, tag="fpos")
        zq_col = per.tile([P, NT], F32, tag="zq")
        qn16 = per.tile([NT, P], s_dt, tag="qn16")

        nc.vector.memset(negone[:], -1.0)
        nc.vector.memset(meps[:], -EPS)
        nc.vector.memset(peps[:], 1.0 + EPS)
        nc.vector.memset(ones64[:], 1.0)
        for c in range(KC):
            nc.sync.dma_start(wq_sb[:, c * DH:(c + 1) * DH],
                              wq_d[c * P:(c + 1) * P, :])
            nc.sync.dma_start(wkv_sb[:, c * KV:(c + 1) * KV],
                              wkv_d[c * P:(c + 1) * P, :])
        nc.sync.dma_start(wo_sb[:], wo_d)
        if cfg["has_bq"]:
            bq_sb = per.tile([DH, 1], F32, tag="bq")
            nc.sync.dma_start(bq_sb[:], bq_d)

        def psum_copy(engine, dst, src):
            if engine == "vector":
                nc.vector.tensor_copy(dst, src)
            else:
                nc.scalar.activation(dst, src, AF.Copy)

        # ---- phase A: x^T arrives pre-transposed from the host ----
        for j in range(KC):
            nc.sync.dma_start(xT[j][:], xt_d[j * P:(j + 1) * P, :])

        # ---- phase B1: merged K|V projections, K-side stats ----
        has_bkv = cfg["has_bkv"]
        with tc.tile_pool(name="kb", bufs=3) as kb, \
             tc.tile_pool(name="psk", bufs=3, space="PSUM") as psk, \
             tc.tile_pool(name="pskt", bufs=2, space="PSUM") as pskt:
            if not has_bkv:  # ones column of v_aug written directly
                nc.vector.memset(
                    vaug[:].rearrange("p (t c) -> p t c", c=DH + 1)[:, :, DH:DH + 1],
                    1.0)
            for g in range(NT // 4):
                ps_t = pskt.tile([DH, 4 * P], s_dt, tag="pskt")
                for q4 in range(4):
                    i = g * 4 + q4
                    ps_kv = psk.tile([P, KV], F32, tag="psk")
                    for c in range(KC):
                        nc.tensor.matmul(
                            ps_kv[:], xT[c][:, i * P:(i + 1) * P],
                            wkv_sb[:, c * KV:(c + 1) * KV],
                            start=(c == 0),
                            stop=(c == KC - 1 and not has_bkv))
                    if has_bkv:
                        nc.tensor.matmul(ps_kv[:], ones1[:], bkv_sb[:],
                                         start=False, stop=True)
                        nc.scalar.activation(
                            vaug[:, i * (DH + 1):(i + 1) * (DH + 1)],
                            ps_kv[:, DH:KV], AF.Copy)
                    else:
                        nc.scalar.activation(
                            vaug[:, i * (DH + 1):i * (DH + 1) + DH],
                            ps_kv[:, DH:DH + DH], AF.Copy)
                    ksq = kb.tile([P, DH], F32, tag="ksq")
                    nc.scalar.activation(ksq[:], ps_kv[:, 0:DH], AF.Square)
                    nc.vector.tensor_reduce(
                        kn_col[:, i:i + 1], ksq[:],
                        axis=mybir.AxisListType.X, op=ALU.add)
                    nc.vector.tensor_scalar(
                        zb_col[:, i:i + 1], kn_col[:, i:i + 1], -1.0, 1.0,
                        ALU.mult, ALU.add)
                    nc.vector.reciprocal(sc_pack[:, i:i + 1],
                                         zb_col[:, i:i + 1])
                    nc.vector.tensor_tensor(
                        sc_pack[:, NT + i:NT + i + 1], sc_pack[:, i:i + 1],
                        kn_col[:, i:i + 1], op=ALU.mult)
                    nc.vector.tensor_scalar_mul(
                        rb2_col[:, i:i + 1], sc_pack[:, i:i + 1], -2.0)
                    kp = kb.tile([P, DH], s_dt, tag="kp")
                    nc.scalar.activation(kp[:], ps_kv[:, 0:DH], AF.Copy,
                                         scale=rb2_col[:, i:i + 1])
                    nc.tensor.transpose(ps_t[:, q4 * P:(q4 + 1) * P],
                                        kp[:], idents[s_dt][:])
                nc.vector.tensor_copy(raug[0:DH, g * 4 * P:(g + 1) * 4 * P],
                                      ps_t[:])
            # stat rows: transpose sc_pack -> [32,128] -> SBUF -> DMA to raug
            ps_sc = pskt.tile([2 * NT, P], F32, tag="pssc")
            nc.tensor.transpose(ps_sc[:], sc_pack[:], ident[:])
            sc_t = kb.tile([2 * NT, P], s_dt, tag="sc_t")
            nc.vector.tensor_copy(sc_t[:], ps_sc[:])
            nc.sync.dma_start(
                raug[DH:DH + 2, :].rearrange("p (t n) -> p t n", n=P),
                sc_t[:])

        # ---- phase B2: Q side (wide) ----
        with tc.tile_pool(name="psq", bufs=2, space="PSUM") as psq, \
             tc.tile_pool(name="psqn", bufs=2, space="PSUM") as psqn, \
             tc.tile_pool(name="qtmp", bufs=1) as qtmp:
            CH = 512
            qnrow = qtmp.tile([2, N], s_dt, tag="qnrow")
            nc.vector.memset(qnrow[:], 1.0)
            for ch in range(N // CH):
                ps_q = psq.tile([DH, CH], F32, tag="psq")
                for c in range(KC):
                    nc.tensor.matmul(
                        ps_q[:], wq_sb[:, c * DH:(c + 1) * DH],
                        xT[c][:, ch * CH:(ch + 1) * CH],
                        start=(c == 0), stop=(c == KC - 1))
                if cfg["has_bq"]:
                    nc.scalar.activation(laug[0:DH, ch * CH:(ch + 1) * CH],
                                         ps_q[:], AF.Identity,
                                         bias=bq_sb[:, 0:1])
                    nc.scalar.activation(qsq[:, ch * CH:(ch + 1) * CH],
                                         ps_q[:], AF.Square,
                                         bias=bq_sb[:, 0:1])
                else:
                    nc.scalar.activation(laug[0:DH, ch * CH:(ch + 1) * CH],
                                         ps_q[:], AF.Copy)
                    nc.scalar.activation(qsq[:, ch * CH:(ch + 1) * CH],
                                         ps_q[:], AF.Square)
                ps_n = psqn.tile([1, CH], F32, tag="psqn")
                nc.tensor.matmul(ps_n[:], ones64[:],
                                 qsq[:, ch * CH:(ch + 1) * CH],
                                 start=True, stop=True)
                nc.vector.tensor_copy(qnrow[0:1, ch * CH:(ch + 1) * CH],
                                      ps_n[:])
            nc.sync.dma_start(laug[DH:DH + 2, :], qnrow[:])
            # qn col-form -> fneg scales
            nc.sync.dma_start(qn16[:], qnrow[0:1, :])
            ps_qc = psqn.tile([P, NT], s_dt, tag="psqc")
            nc.tensor.transpose(ps_qc[:], qn16[:], idents[s_dt][0:NT, 0:NT])
            nc.vector.tensor_scalar(zq_col[:], ps_qc[:], -1.0, 1.0,
                                    ALU.mult, ALU.add)
            nc.vector.reciprocal(fpos_col[:], zq_col[:])
            nc.vector.tensor_scalar_mul(fneg_col[:], fpos_col[:], -2.0)
            nc.vector.tensor_scalar_mul(fpos_col[:], fpos_col[:], 2.0)

        # ---- phase C: rows ----
        clip = cfg["clip_path"]
        hm_dt = F16 if cfg["hmin16"] else F32
        with tc.tile_pool(name="hp", bufs=cfg["hp_bufs"]) as hpp, \
             tc.tile_pool(name="cwrow", bufs=cfg["row_bufs"]) as cwrow, \
             tc.tile_pool(name="cw", bufs=cfg["cw_bufs"]) as cw, \
             tc.tile_pool(name="etp", bufs=cfg["et_bufs"]) as etp, \
             tc.tile_pool(name="stat", bufs=2) as statp, \
             tc.tile_pool(name="pss", bufs=3, space="PSUM") as pss, \
             tc.tile_pool(name="pset", bufs=2, space="PSUM") as pset, \
             tc.tile_pool(name="pso", bufs=2, space="PSUM") as pso, \
             tc.tile_pool(name="psop", bufs=1, space="PSUM") as psop:
            def stage1(r):
                """Scores + elementwise chain for row r -> (hp, rh)."""
                W = (r + 1) * P
                hp = hpp.tile([P, N], hp_dt, tag="hp", name=f"hp{r}")
                hmin = statp.tile([P, NT], hm_dt, tag="hmin",
                                  name=f"hmin{r}")
                f_ap = fneg_col[:, r:r + 1]
                fp_ap = fpos_col[:, r:r + 1]
                if clip == "relu_row":
                    # Relu per chunk (frees PSUM), then row-wide passes.
                    tr = cwrow.tile([P, N], F32, tag="tr")
                    o = 0
                    while o < W:
                        w = min(512, W - o)
                        ps_s = pss.tile([P, 512], F32, tag="pss")
                        nc.tensor.matmul(
                            ps_s[:, 0:w], laug[:, r * P:(r + 1) * P],
                            raug[:, o:o + w], start=True, stop=True)
                        nc.scalar.activation(tr[:, o:o + w], ps_s[:, 0:w],
                                             AF.Relu, bias=meps[:, 0:1],
                                             scale=fp_ap)
                        o += w
                    sq = cwrow.tile([P, N], F32, tag="sqr")
                    nc.scalar.activation(sq[:, 0:W], tr[:, 0:W], AF.Square,
                                         bias=peps[:, 0:1])
                    u_t = cwrow.tile([P, N], F32, tag="ur")
                    nc.scalar.activation(u_t[:, 0:W], sq[:, 0:W], AF.Sqrt,
                                         bias=negone[:, 0:1])
                    # hn = (tr*-1 - (1+eps)) + u  (uses clipped t; diff ~1e-6)
                    nc.vector.affine_then_add(hp[:, 0:W], tr[:, 0:W],
                                              u_t[:, 0:W], -1.0,
                                              -(1.0 + EPS))
                    nc.vector.tensor_tensor(hp[:, W - P:W], hp[:, W - P:W],
                                            trimask[:], op=ALU.mult)
                    nc.vector.tensor_reduce(
                        hmin[:, 0:r + 1],
                        hp[:, 0:W].rearrange("p (b n) -> p b n", n=P),
                        axis=mybir.AxisListType.X, op=ALU.min)
                else:
                  o = 0
                  while o < W:
                    w = min(512, W - o)
                    ps_s = pss.tile([P, 512], F32, tag="pss")
                    nc.tensor.matmul(
                        ps_s[:, 0:w], laug[:, r * P:(r + 1) * P],
                        raug[:, o:o + w], start=True, stop=True)
                    if clip == "relu":
                        tr = cw.tile([P, 512], F32, tag="tr")
                        nc.scalar.activation(tr[:, 0:w], ps_s[:, 0:w],
                                             AF.Relu, bias=meps[:, 0:1],
                                             scale=fp_ap)
                        sq = cw.tile([P, 512], F32, tag="sq")
                        nc.scalar.activation(sq[:, 0:w], tr[:, 0:w],
                                             AF.Square, bias=peps[:, 0:1])
                        u_in = sq
                    else:
                        sq = cw.tile([P, 512], F32, tag="sq")
                        nc.scalar.activation(sq[:, 0:w], ps_s[:, 0:w],
                                             AF.Square, bias=negone[:, 0:1],
                                             scale=f_ap)
                        d_t = cw.tile([P, 512], F32, tag="d")
                        eng = (nc.gpsimd if clip == "gpsimd" else nc.vector)
                        eng.tensor_scalar(d_t[:, 0:w], sq[:, 0:w], -1.0,
                                          CLIP2, ALU.add, ALU.max)
                        u_in = d_t
                    u_t = cw.tile([P, 512], F32, tag="u")
                    nc.scalar.activation(u_t[:, 0:w], u_in[:, 0:w], AF.Sqrt,
                                         bias=(negone[:, 0:1]
                                               if clip == "relu" else 0.0))
                    nc.vector.affine_then_add(hp[:, o:o + w], ps_s[:, 0:w],
                                              u_t[:, 0:w], f_ap, -1.0)
                    if o + w == W:  # diagonal block: zero strict upper tri
                        nc.vector.tensor_tensor(hp[:, W - P:W], hp[:, W - P:W],
                                                trimask[:], op=ALU.mult)
                    nb = w // P
                    nc.vector.tensor_reduce(
                        hmin[:, o // P:o // P + nb],
                        hp[:, o:o + w].rearrange("p (b n) -> p b n", n=P),
                        axis=mybir.AxisListType.X, op=ALU.min)
                    o += w
                rh = statp.tile([P, NT], F32, tag="rh", name=f"rh{r}")
                nc.vector.reciprocal(rh[:, 0:r + 1], hmin[:, 0:r + 1])
                return hp, rh

            def stage2(r, hp, rh):
                """eT + PV + out-proj for row r."""
                W = (r + 1) * P
                ps_o = pso.tile([P, DH + 1], F32, tag="pso")
                nblk = r + 1
                for g in range((nblk + 3) // 4):
                    c0, c1 = g * 4, min(g * 4 + 4, nblk)
                    ps_et = pset.tile([P, 512], F32, tag="pset")
                    et_sb = etp.tile([P, 512], pv_dt, tag="et")
                    for c in range(c0, c1):
                        q = c - c0
                        diag = cw.tile([P, P], hp_dt, tag="diag")
                        nc.gpsimd.affine_select(
                            diag[:], rh[:, c:c + 1].broadcast_to([P, P]),
                            pattern=[[1, P]], compare_op=ALU.is_equal,
                            fill=0.0, base=0, channel_multiplier=-1)
                        nc.tensor.matmul(
                            ps_et[:, q * P:(q + 1) * P],
                            hp[:, c * P:(c + 1) * P], diag[:],
                            start=True, stop=True)
                    wg = (c1 - c0) * P
                    psum_copy(cfg["etcopy_engine"], et_sb[:, 0:wg],
                              ps_et[:, 0:wg])
                    for c in range(c0, c1):
                        q = c - c0
                        nc.tensor.matmul(
                            ps_o[:], et_sb[:, q * P:(q + 1) * P],
                            vaug[:, c * (DH + 1):(c + 1) * (DH + 1)],
                            start=(c == 0), stop=(c == nblk - 1),
                            skip_group_check=True)
                # out-proj on UNNORMALIZED o; 1/norm folded into final copy
                # (per-row scale commutes out of the Dh contraction)
                ns = statp.tile([P, 1], F32, tag="ns")
                nc.vector.tensor_scalar_max(ns[:], ps_o[:, DH:DH + 1], EPS)
                rn = statp.tile([P, 1], F32, tag="rn")
                nc.vector.reciprocal(rn[:], ns[:])
                o_sb = statp.tile([P, DH], op_dt, tag="o_sb")
                nc.scalar.activation(o_sb[:], ps_o[:, 0:DH], AF.Copy)
                ps_ot = pset.tile([DH, P], op_dt, tag="pset")
                nc.tensor.transpose(ps_ot[:], o_sb[:], idents[op_dt][:])
                ot_sb = statp.tile([DH, P], op_dt, tag="ot")
                nc.vector.tensor_copy(ot_sb[:], ps_ot[:])
                ps_op = psop.tile([P, D], F32, tag="psop")
                nc.tensor.matmul(ps_op[:], ot_sb[:], wo_sb[:],
                                 start=True, stop=True)
                op_sb = etp.tile([P, D], F32, tag="op_sb")
                nc.scalar.activation(op_sb[:], ps_op[:], AF.Copy, scale=rn[:])
                nc.sync.dma_start(out_d[r * P:(r + 1) * P, :], op_sb[:])

            if cfg["pipeline"]:
                state = {}
                for r in range(NT + 1):
                    if r < NT:
                        state[r] = stage1(r)
                    if r >= 1:
                        stage2(r - 1, *state.pop(r - 1))
            else:
                for r in range(NT):
                    hp_r, rh_r = stage1(r)
                    stage2(r, hp_r, rh_r)

    nc.compile()
    return nc, names


def _host_fallback(x, c, Wq, bq, Wk, bk, Wv, bv, Wo, bo):
    """Numpy replica of the reference for inputs outside the specialized
    regime (never hit for the shipped setup_inputs). Kept for safety."""
    B, N_, D_ = x.shape
    Dh = D_ // H
    cc = np.maximum(np.abs(c), 1e-6)[0]
    sqrt_c = np.sqrt(max(cc, EPS))
    x2 = x.reshape(N_, D_)

    def proj(W, b):
        return (x2 @ W + b).reshape(N_, H, Dh).transpose(1, 0, 2)

    q, k, v = proj(Wq, bq), proj(Wk, bk), proj(Wv, bv)
    qn = (q ** 2).sum(-1)
    kn = (k ** 2).sum(-1)
    out = np.zeros((H, N_, Dh), np.float32)
    BM = P
    for h in range(H):
        qk = q[h] @ k[h].T
        diff = np.clip(qn[h][:, None] + kn[h][None, :] - 2 * qk, 0, None)
        den = np.clip((1 - cc * qn[h])[:, None] * (1 - cc * kn[h])[None, :],
                      EPS, None)
        arg = np.clip(1 + 2 * cc * diff / den, 1 + EPS, None)
        s = -np.arccosh(arg) / sqrt_c
        nbm = N_ // BM
        tri = np.triu(np.ones((BM, BM), bool), 1)
        e = np.zeros_like(s)
        for r in range(nbm):
            for cb in range(r + 1):
                blk = s[r * BM:(r + 1) * BM, cb * BM:(cb + 1) * BM].copy()
                m = tri if cb == r else np.zeros((BM, BM), bool)
                bm = np.where(m, -np.inf, blk).max(axis=1, keepdims=True)
                bm = np.where(np.isfinite(bm), bm, 0.0)
                eb = np.where(m, 0.0, np.exp(blk - bm))
                e[r * BM:(r + 1) * BM, cb * BM:(cb + 1) * BM] = eb
        norm = np.clip(e.sum(axis=1), EPS, None)
        out[h] = (e @ v[h]) / norm[:, None]
    full = out.transpose(1, 0, 2).reshape(N_, D_)
    return (full @ Wo + bo).reshape(B, N_, D_).astype(np.float32)


_PROG_CACHE = {}


def _get_program(cfg_key, cfg):
    if cfg_key not in _PROG_CACHE:
        _PROG_CACHE[cfg_key] = build_program(cfg)
    return _PROG_CACHE[cfg_key]


def make_in_maps(x, Wq, bq, Wk, bk, Wv, bv, Wo, cfg=None):
    cfg = {**DEFAULT_CFG, **(cfg or {})}
    np_pj = _npdt(cfg["proj_dt"])
    np_op = _npdt(cfg["op_dt"])
    np_tm = _npdt(cfg["et_dt"])
    xt = np.ascontiguousarray(
        x.reshape(N, D).astype(np.float32).T).astype(np_pj)
    ident = np.eye(P, dtype=np.float32)
    trimask = np.tril(np.ones((P, P), np.float32)).astype(np_tm)
    in_maps = []
    for h in range(H):
        sl = slice(h * DH, (h + 1) * DH)
        wkv = np.zeros((D, KV), np.float32)
        wkv[:, :DH] = Wk[:, sl]
        wkv[:, DH:DH + DH] = Wv[:, sl]
        m = {
            "xt": xt,
            "wq": np.ascontiguousarray(Wq[:, sl]).astype(np_pj),
            "wkv": wkv.astype(np_pj),
            "wo": np.ascontiguousarray(Wo[sl, :]).astype(np_op),
            "ident": ident,
            "trimask": trimask,
        }
        if cfg["has_bkv"]:
            bkv = np.zeros((1, KV), np.float32)
            bkv[0, :DH] = bk[sl]
            bkv[0, DH:DH + DH] = bv[sl]
            bkv[0, DH + DH] = 1.0
            m["bkvrow"] = bkv.astype(np_pj)
        if cfg["has_bq"]:
            m["bqcol"] = np.ascontiguousarray(
                bq[sl].reshape(DH, 1)).astype(np.float32)
        in_maps.append(m)
    return in_maps


def run_device(x, Wq, bq, Wk, bk, Wv, bv, Wo, cfg=None, trace=False,
               tmpdir=None):
    cfg_full = {**DEFAULT_CFG, **(cfg or {})}
    cfg_full["has_bq"] = bool(np.any(bq))
    cfg_full["has_bkv"] = bool(np.any(bk)) or bool(np.any(bv))
    cfg_key = tuple(sorted(cfg_full.items()))
    nc, _ = _get_program(cfg_key, cfg_full)
    in_maps = make_in_maps(x, Wq, bq, Wk, bk, Wv, bv, Wo, cfg_full)
    res = run_bass_kernel_spmd(nc, in_maps, core_ids=list(range(H)),
                               trace=trace, tmpdir=tmpdir)
    partial = np.zeros((N, D), np.float64)
    for rm in res.results:
        partial += rm["out"].astype(np.float64)
    return partial, res


def kernel(x, c, Wq, bq, Wk, bk, Wv, bv, Wo, bo):
    x = np.asarray(x); c = np.asarray(c)
    Wq = np.asarray(Wq, np.float32); bq = np.asarray(bq, np.float32)
    Wk = np.asarray(Wk, np.float32); bk = np.asarray(bk, np.float32)
    Wv = np.asarray(Wv, np.float32); bv = np.asarray(bv, np.float32)
    Wo = np.asarray(Wo, np.float32); bo = np.asarray(bo, np.float32)

    cc = max(abs(float(c.reshape(-1)[0])), 1e-6)
    if abs(np.sqrt(max(cc, EPS)) - 1.0) > 1e-9:
        return _host_fallback(x, c, Wq, bq, Wk, bk, Wv, bv, Wo, bo)

    partial, _ = run_device(x, Wq, bq, Wk, bk, Wv, bv, Wo)
    out = (partial + bo.astype(np.float64)).astype(np.float32)
    return out.reshape(1, N, D)
